# revision 3
# baseline (speedup 1.0000x reference)
"""Trainium2 Bass kernel for a custom GRU cell.

    x_h   = concat([inputs, h_prev], -1)            # [B, D+U]
    z     = sigmoid(x_h @ Wz)                       # [B, U]
    r     = sigmoid(x_h @ Wr)                       # [B, U]
    h_hat = tanh(concat([inputs, r * h_prev]) @ Wh) # [B, U]
    out   = z * h_prev + (1 - z) * h_hat

Data-parallel over 8 NeuronCores: batch dim sharded, weights replicated.

Per-core layout (B_c = 2048 rows):
  - weights DMA'd once, feature-major [128, chunk, 512], dtype f32r
  - per 128-row batch tile: PE-transpose inputs/h_prev (and r*h_prev)
    128x128 blocks into feature-major f32r lhsT tiles, then 8-chunk
    accumulated f32r matmuls per gate (out = xT.T @ W_chunk in PSUM),
    sigmoid/tanh on ScalarE straight out of PSUM, gate combine on
    VectorE, DMA out. Emission is software-pipelined (gate-h work of
    tile i is emitted after the z/r work of tile i+1) to keep the PE
    dense across the ACT/DVE dependency gap.
"""

import sys

for _p in ("/opt/trn_rl_repo", "/root/.axon_site/_ro/trn_rl_repo"):
    if _p not in sys.path:
        sys.path.append(_p)

import numpy as np

B, D, U = 16384, 512, 512
K = D + U
N_CORES = 8
BC = B // N_CORES          # rows per core
PT = 128                   # partition tile (batch rows per tile)
NT = BC // PT              # batch tiles per core
KC = K // 128              # contraction chunks (8)
DC = D // 128              # chunks coming from `inputs` (4)


def build_gru_tile_kernel(tc, d_in, d_hp, d_wz, d_wr, d_wh, d_out, nt=NT):
    """Emit the GRU cell body into TileContext `tc`.

    d_* are DRAM APs: d_in/d_hp [nt*128, 512], d_w* [1024, 512],
    d_out [nt*128, 512].
    """
    import concourse.bass as bass
    import concourse.tile as tile
    from concourse import mybir
    from concourse.masks import make_identity

    f32 = mybir.dt.float32
    f32r = mybir.dt.float32r
    nc = tc.nc
    ctx = tc.nc  # noqa

    Sig = mybir.ActivationFunctionType.Sigmoid
    Tanh = mybir.ActivationFunctionType.Tanh
    Alu = mybir.AluOpType

    import contextlib

    est = contextlib.ExitStack()
    sing = est.enter_context(tc.tile_pool(name="sing", bufs=1))
    wpool = est.enter_context(tc.tile_pool(name="w", bufs=1))
    io = est.enter_context(tc.tile_pool(name="io", bufs=3))
    xtp = est.enter_context(tc.tile_pool(name="xtp", bufs=3))
    actp = est.enter_context(tc.tile_pool(name="act", bufs=3))
    # PSUM: 8 banks total. One shared tag per pool so slots rotate:
    # 3 transpose banks + 4 gate banks = 7.
    pst = est.enter_context(tc.tile_pool(name="pst", bufs=3, space="PSUM"))
    psg = est.enter_context(tc.tile_pool(name="psg", bufs=4, space="PSUM"))

    ident = sing.tile([128, 128], f32)
    make_identity(nc, ident)

    # Weights, feature-major chunks: [128 part, chunk, 512], f32r.
    w_sb = {}
    for name, dram in (("wz", d_wz), ("wr", d_wr), ("wh", d_wh)):
        t = wpool.tile([128, KC, 512], f32r, tag=name)
        nc.sync.dma_start(t[:], dram.rearrange("(c p) n -> p c n", p=128).bitcast(f32r))
        w_sb[name] = t

    def transpose4(src, tag):
        """PE-transpose four 128x128 blocks of `src` [128, 512] into a
        feature-major f32r SBUF tile [128, 512] (chunks side by side)."""
        ps = pst.tile([128, 512], f32, tag="pstr")
        for k in range(4):
            nc.tensor.transpose(ps[:, 128 * k : 128 * (k + 1)],
                                src[:, 128 * k : 128 * (k + 1)], ident[:])
        sb = xtp.tile([128, 512], f32r, tag=tag)
        nc.scalar.copy(sb[:], ps[:])
        return sb

    def gate_mm(lhs_lo, lhs_hi, w, tag):
        """Accumulate 8-chunk matmul into a PSUM tile [128, 512]."""
        ps = psg.tile([128, 512], f32, tag="psg")
        for k in range(KC):
            lhs = lhs_lo if k < DC else lhs_hi
            kk = k % 4
            nc.tensor.matmul(ps[:], lhs[:, 128 * kk : 128 * (kk + 1)],
                             w[:, k, :], start=(k == 0), stop=(k == KC - 1))
        return ps

    # Per-tile state carried between the two pipeline phases.
    state = [None] * nt

    def phase_zr(i):
        xin = io.tile([128, 512], f32, tag="xin")
        nc.sync.dma_start(xin[:], d_in[128 * i : 128 * (i + 1), :])
        hp = io.tile([128, 512], f32, tag="hp")
        nc.sync.dma_start(hp[:], d_hp[128 * i : 128 * (i + 1), :])

        xT = transpose4(xin, "xT")
        hT = transpose4(hp, "hT")

        ps_z = gate_mm(xT, hT, w_sb["wz"], "ps_z")
        ps_r = gate_mm(xT, hT, w_sb["wr"], "ps_r")

        z_s = actp.tile([128, 512], f32, tag="z_s")
        nc.scalar.activation(z_s[:], ps_z[:], Sig)
        r_s = actp.tile([128, 512], f32, tag="r_s")
        nc.scalar.activation(r_s[:], ps_r[:], Sig)

        rh = actp.tile([128, 512], f32, tag="rh")
        nc.vector.tensor_mul(rh[:], r_s[:], hp[:])

        state[i] = (xT, hp, z_s, rh)

    def phase_h(i):
        xT, hp, z_s, rh = state[i]
        rhT = transpose4(rh, "rhT")
        ps_h = gate_mm(xT, rhT, w_sb["wh"], "ps_h")

        hh = actp.tile([128, 512], f32, tag="hh")
        nc.scalar.activation(hh[:], ps_h[:], Tanh)

        # out = hh + z * (hp - hh)
        t = actp.tile([128, 512], f32, tag="t")
        nc.vector.tensor_sub(t[:], hp[:], hh[:])
        t2 = actp.tile([128, 512], f32, tag="t2")
        nc.vector.tensor_mul(t2[:], z_s[:], t[:])
        out = actp.tile([128, 512], f32, tag="out")
        nc.vector.tensor_add(out[:], t2[:], hh[:])
        nc.sync.dma_start(d_out[128 * i : 128 * (i + 1), :], out[:])
        state[i] = None

    # Software pipeline: h-phase trails the z/r-phase by one tile.
    phase_zr(0)
    for i in range(1, nt):
        phase_zr(i)
        phase_h(i - 1)
    phase_h(nt - 1)

    est.close()


_NC_CACHE = {}


def _build(nt=NT):
    if nt in _NC_CACHE:
        return _NC_CACHE[nt]
    import concourse.tile as tile
    from concourse import bacc, mybir

    f32 = mybir.dt.float32
    nc = bacc.Bacc("TRN2", target_bir_lowering=False, debug=False)
    d_in = nc.dram_tensor("inputs", [nt * PT, D], f32, kind="ExternalInput").ap()
    d_hp = nc.dram_tensor("h_prev", [nt * PT, U], f32, kind="ExternalInput").ap()
    d_wz = nc.dram_tensor("Wz", [K, U], f32, kind="ExternalInput").ap()
    d_wr = nc.dram_tensor("Wr", [K, U], f32, kind="ExternalInput").ap()
    d_wh = nc.dram_tensor("Wh", [K, U], f32, kind="ExternalInput").ap()
    d_out = nc.dram_tensor("out", [nt * PT, U], f32, kind="ExternalOutput").ap()

    with tile.TileContext(nc) as tc:
        build_gru_tile_kernel(tc, d_in, d_hp, d_wz, d_wr, d_wh, d_out, nt=nt)
    nc.compile()
    _NC_CACHE[nt] = nc
    return nc


def run_sharded(inputs, h_prev, Wz, Wr, Wh, trace=False):
    from concourse.bass_utils import run_bass_kernel_spmd

    nc = _build()
    inputs = np.ascontiguousarray(np.asarray(inputs, dtype=np.float32))
    h_prev = np.ascontiguousarray(np.asarray(h_prev, dtype=np.float32))
    Wz = np.ascontiguousarray(np.asarray(Wz, dtype=np.float32))
    Wr = np.ascontiguousarray(np.asarray(Wr, dtype=np.float32))
    Wh = np.ascontiguousarray(np.asarray(Wh, dtype=np.float32))
    in_maps = [
        {
            "inputs": inputs[i * BC : (i + 1) * BC],
            "h_prev": h_prev[i * BC : (i + 1) * BC],
            "Wz": Wz,
            "Wr": Wr,
            "Wh": Wh,
        }
        for i in range(N_CORES)
    ]
    res = run_bass_kernel_spmd(
        nc, in_maps, core_ids=list(range(N_CORES)), trace=trace
    )
    out = np.concatenate([res.results[i]["out"] for i in range(N_CORES)], axis=0)
    return out, res


def kernel(inputs, h_prev, Wz, Wr, Wh):
    out, _ = run_sharded(inputs, h_prev, Wz, Wr, Wh, trace=False)
    return out
